# revision 1
# baseline (speedup 1.0000x reference)
"""GroupedSwiGLU MoE kernel for 8x Trainium2 NeuronCores.

Strategy: expert-parallel. Core e gets expert e's weights and its (padded)
token group. Inside each core:
  phase 1: gateT/upT[inter, tok] = Wg/Wu^T-contracted matmuls vs xT
  swiglu : hT = silu(gateT) * upT
  phase 2: out[tok, hid] = hT^T-contracted matmuls vs Wd, scaled by probs
All matmul operands bf16 (fp32 PSUM accumulate); host does the
transpose/tiling/padding and the final gather.
"""

import numpy as np
import ml_dtypes
from contextlib import ExitStack

import concourse.bass as bass
import concourse.mybir as mybir
import concourse.tile as tile
from concourse.bacc import Bacc
from concourse.bass_utils import run_bass_kernel_spmd

E = 8
HID = 2048
INTER = 1408
P = 128
KO_H = HID // P    # 16 k-tiles for phase-1 contraction
KO_I = INTER // P  # 11 k-tiles for phase-2 contraction / m-tiles in phase 1
TF = 512           # phase-1 moving free chunk (tokens)
NF = 512           # phase-2 moving free chunk (hid)

F32 = mybir.dt.float32
BF16 = mybir.dt.bfloat16
NP_BF16 = ml_dtypes.bfloat16

_nc_cache: dict = {}


def _build(T: int):
    """Per-core Bass program for T padded tokens (T % 512 == 0)."""
    nc = Bacc()
    xT = nc.dram_tensor("xT", [P, KO_H, T], BF16, kind="ExternalInput")
    wg = nc.dram_tensor("wg", [P, KO_I, KO_H, P], BF16, kind="ExternalInput")
    wu = nc.dram_tensor("wu", [P, KO_I, KO_H, P], BF16, kind="ExternalInput")
    wd = nc.dram_tensor("wd", [P, KO_I, HID], BF16, kind="ExternalInput")
    probs = nc.dram_tensor("probs", [P, T // P], F32, kind="ExternalInput")
    out = nc.dram_tensor("out", [T, HID], F32, kind="ExternalOutput")

    n_tf = T // TF
    n_t = T // P
    n_nf = HID // NF

    with tile.TileContext(nc) as tc, ExitStack() as ctx:
        resident = ctx.enter_context(tc.tile_pool(name="resident", bufs=1))
        wpool = ctx.enter_context(tc.tile_pool(name="weights", bufs=2))
        tmp = ctx.enter_context(tc.tile_pool(name="tmp", bufs=3))
        opool = ctx.enter_context(tc.tile_pool(name="outp", bufs=4))
        psum = ctx.enter_context(tc.tile_pool(name="psum", bufs=2, space="PSUM"))

        xT_sb = resident.tile([P, KO_H, T], BF16)
        for k in range(KO_H):
            nc.sync.dma_start(xT_sb[:, k], xT[:, k])
        wd_sb = resident.tile([P, KO_I, HID], BF16)
        for k in range(KO_I):
            nc.sync.dma_start(wd_sb[:, k], wd[:, k])
        probs_dma = resident.tile([P, T // P], F32)
        nc.sync.dma_start(probs_dma[:], probs[:])
        # Bounce through DVE so phase-2 scaling (DVE) only ever needs the PE
        # wait: the TensorScalar ISA slot can't carry a second (DMA) wait.
        probs_sb = resident.tile([P, T // P], F32)
        nc.vector.tensor_copy(probs_sb[:], probs_dma[:])
        hT_sb = resident.tile([P, KO_I, T], BF16)

        # Phase 1: for each inter m-tile, gateT/upT psum then fused silu*mul
        for m in range(KO_I):
            wg_m = wpool.tile([P, KO_H, P], BF16, tag="wg")
            nc.gpsimd.dma_start(wg_m[:], wg[:, m])
            wu_m = wpool.tile([P, KO_H, P], BF16, tag="wu")
            nc.gpsimd.dma_start(wu_m[:], wu[:, m])
            for f in range(n_tf):
                pg = psum.tile([P, TF], F32, tag="pg")
                pu = psum.tile([P, TF], F32, tag="pu")
                for k in range(KO_H):
                    nc.tensor.matmul(
                        pg[:], wg_m[:, k], xT_sb[:, k, bass.ts(f, TF)],
                        start=(k == 0), stop=(k == KO_H - 1),
                    )
                for k in range(KO_H):
                    nc.tensor.matmul(
                        pu[:], wu_m[:, k], xT_sb[:, k, bass.ts(f, TF)],
                        start=(k == 0), stop=(k == KO_H - 1),
                    )
                sg = tmp.tile([P, TF], F32, tag="sg")
                nc.scalar.activation(
                    sg[:], pg[:], mybir.ActivationFunctionType.Silu
                )
                # ACT copy of up-psum so the DVE mul has a single-engine wait
                su = tmp.tile([P, TF], F32, tag="su")
                nc.scalar.copy(su[:], pu[:])
                nc.vector.tensor_mul(
                    hT_sb[:, m, bass.ts(f, TF)], sg[:], su[:]
                )

        # Phase 2: out tiles [128 tok, 512 hid], contract over inter
        for t in range(n_t):
            for n in range(n_nf):
                po = psum.tile([P, NF], F32, tag="po")
                for k in range(KO_I):
                    nc.tensor.matmul(
                        po[:], hT_sb[:, k, bass.ts(t, P)],
                        wd_sb[:, k, bass.ts(n, NF)],
                        start=(k == 0), stop=(k == KO_I - 1),
                    )
                ot = opool.tile([P, NF], F32, tag="ot")
                nc.vector.tensor_scalar_mul(ot[:], po[:], probs_sb[:, t : t + 1])
                nc.sync.dma_start(out[bass.ts(t, P), bass.ts(n, NF)], ot[:])
    nc.finalize()
    return nc


def _pack_core(x_pad, probs_pad, wg_e, wu_e, wd_e, T):
    """Host-side tiling into the DRAM layouts the kernel expects."""
    # xT[p, k, t] = x_pad[t, k*128+p]
    xT = np.ascontiguousarray(
        x_pad.T.reshape(KO_H, P, T).transpose(1, 0, 2)
    ).astype(NP_BF16)
    # wg[p, m, k, i] = w_gate[k*128+p, m*128+i]
    wgt = np.ascontiguousarray(
        wg_e.reshape(KO_H, P, KO_I, P).transpose(1, 2, 0, 3)
    ).astype(NP_BF16)
    wut = np.ascontiguousarray(
        wu_e.reshape(KO_H, P, KO_I, P).transpose(1, 2, 0, 3)
    ).astype(NP_BF16)
    # wd[p, k, h] = w_down[k*128+p, h]
    wdt = np.ascontiguousarray(
        wd_e.reshape(KO_I, P, HID).transpose(1, 0, 2)
    ).astype(NP_BF16)
    # probs[p, o] = probs_pad[o*128+p]
    pr = np.ascontiguousarray(probs_pad.reshape(T // P, P).T).astype(np.float32)
    return {"xT": xT, "wg": wgt, "wu": wut, "wd": wdt, "probs": pr}


def _run(inputs, trace=False):
    x = np.asarray(inputs["permuted_x"], np.float32)
    probs = np.asarray(inputs["permuted_probs"], np.float32)
    wg = np.asarray(inputs["w_gate"], np.float32)
    wu = np.asarray(inputs["w_up"], np.float32)
    wd = np.asarray(inputs["w_down"], np.float32)
    counts = np.asarray(inputs["tokens_per_expert"]).astype(np.int64)
    offs = np.concatenate([[0], np.cumsum(counts)])
    assert offs[-1] == x.shape[0]

    T = int(max(1, counts.max()))
    T = ((T + 511) // 512) * 512

    key = T
    if key not in _nc_cache:
        _nc_cache[key] = _build(T)
    nc = _nc_cache[key]

    in_maps = []
    for e in range(E):
        n = int(counts[e])
        s = int(offs[e])
        x_pad = np.zeros((T, HID), np.float32)
        x_pad[:n] = x[s : s + n]
        p_pad = np.zeros((T,), np.float32)
        p_pad[:n] = probs[s : s + n]
        in_maps.append(_pack_core(x_pad, p_pad, wg[e], wu[e], wd[e], T))

    res = run_bass_kernel_spmd(nc, in_maps, core_ids=list(range(E)), trace=trace)

    y = np.empty((x.shape[0], HID), np.float32)
    for e in range(E):
        n = int(counts[e])
        s = int(offs[e])
        y[s : s + n] = res.results[e]["out"][:n]
    return y, res


def kernel(**inputs) -> np.ndarray:
    y, _ = _run(inputs, trace=False)
    return y



# revision 7
# speedup vs baseline: 1.2950x; 1.2950x over previous
"""GroupedSwiGLU MoE kernel for 8x Trainium2 NeuronCores.

Strategy: load-balanced expert-parallel. The SPMD constraint requires one
program for all cores, so each core runs a fixed list of token *segments*
(e.g. 384+384+256 = 1024 tokens), each segment expert-pure with its own
full weight-set input. The host solves an exact packing of the ragged
per-expert token counts into the 8x per-core slot structure (zero padding
for the reference counts), so every core does 1/8 of the FLOPs instead of
padding to the largest expert.

Inside each core, per segment:
  phase 1: gateT/upT[inter, tok] = Wg/Wu^T-contracted matmuls vs xT
  swiglu : hT = silu(gateT) * upT
  phase 2: out[tok, hid] = hT^T-contracted matmuls vs Wd, scaled by probs
All matmul operands bf16 (fp32 PSUM accumulate); host does the
transpose/tiling/padding and the final gather.
"""

import numpy as np
import ml_dtypes
from contextlib import ExitStack

import concourse.bass as bass
import concourse.mybir as mybir
import concourse.tile as tile
from concourse.bacc import Bacc
from concourse.bass_utils import run_bass_kernel_spmd

E = 8
HID = 2048
INTER = 1408
P = 128
KO_H = HID // P    # 16 k-tiles for phase-1 contraction
KO_I = INTER // P  # 11 k-tiles for phase-2 contraction / m-tiles in phase 1
NF = 512           # phase-2 moving free chunk (hid)

F32 = mybir.dt.float32
BF16 = mybir.dt.bfloat16
NP_BF16 = ml_dtypes.bfloat16

_nc_cache: dict = {}


def _build(segs: tuple):
    """Per-core Bass program for segment sizes `segs` (each a multiple of
    128). Segment i contracts against its own weight set wg{i}/wu{i}/wd{i}."""
    nc = Bacc()
    T = sum(segs)
    n_t = T // P
    n_nf = HID // NF

    xT = nc.dram_tensor("xT", [P, KO_H, T], BF16, kind="ExternalInput")
    wgs = [nc.dram_tensor(f"wg{s}", [P, KO_I, KO_H, P], BF16, kind="ExternalInput")
           for s in range(len(segs))]
    wus = [nc.dram_tensor(f"wu{s}", [P, KO_I, KO_H, P], BF16, kind="ExternalInput")
           for s in range(len(segs))]
    wds = [nc.dram_tensor(f"wd{s}", [P, KO_I, HID], BF16, kind="ExternalInput")
           for s in range(len(segs))]
    probs = nc.dram_tensor("probs", [P, n_t], F32, kind="ExternalInput")
    out = nc.dram_tensor("out", [T, HID], BF16, kind="ExternalOutput")

    # DMA queue assignment: three independent DGE rings so the two weight
    # streams and the bulk x/wd/out traffic don't round-robin against each
    # other mid-phase.  wg -> SP (sync), wu -> SWDGE (gpsimd),
    # xT/probs/wd -> ACT (scalar), out -> SP (fills sync's idle ph2 window).
    with tile.TileContext(nc) as tc, ExitStack() as ctx:
        resident = ctx.enter_context(tc.tile_pool(name="resident", bufs=1))
        wpool = ctx.enter_context(tc.tile_pool(name="weights", bufs=2))
        wdpool = ctx.enter_context(tc.tile_pool(name="wdown", bufs=1))
        tmp = ctx.enter_context(tc.tile_pool(name="tmp", bufs=3))
        opool = ctx.enter_context(tc.tile_pool(name="outp", bufs=4))
        psum = ctx.enter_context(tc.tile_pool(name="psum", bufs=3, space="PSUM"))
        psum2 = ctx.enter_context(tc.tile_pool(name="psum2", bufs=2, space="PSUM"))

        # First matmul needs wg0[m0] + wu0[m0] + xT[k0] — issue those DMAs
        # first on their rings so the SDMA engines serve them immediately.
        wg_m0 = wpool.tile([P, KO_H, P], BF16, tag="wg")
        nc.sync.dma_start(wg_m0[:], wgs[0][:, 0])
        wu_m0 = wpool.tile([P, KO_H, P], BF16, tag="wu")
        nc.gpsimd.dma_start(wu_m0[:], wus[0][:, 0])

        xT_sb = resident.tile([P, KO_H, T], BF16)
        for k in range(KO_H):
            nc.scalar.dma_start(xT_sb[:, k], xT[:, k])
        probs_dma = resident.tile([P, n_t], F32)
        nc.scalar.dma_start(probs_dma[:], probs[:])
        # Bounce through DVE so phase-2 scaling (DVE) only ever needs the PE
        # wait: the TensorScalar ISA slot can't carry a second (DMA) wait.
        probs_sb = resident.tile([P, n_t], F32)
        nc.vector.tensor_copy(probs_sb[:], probs_dma[:])
        hT_sb = resident.tile([P, KO_I, T], BF16)

        off = 0
        for s, ntok in enumerate(segs):
            # Phase 1: for each inter m-tile, gateT/upT psum then silu*mul
            for m in range(KO_I):
                if s == 0 and m == 0:
                    wg_m, wu_m = wg_m0, wu_m0
                else:
                    wg_m = wpool.tile([P, KO_H, P], BF16, tag="wg")
                    nc.sync.dma_start(wg_m[:], wgs[s][:, m])
                    wu_m = wpool.tile([P, KO_H, P], BF16, tag="wu")
                    nc.gpsimd.dma_start(wu_m[:], wus[s][:, m])
                pg = psum.tile([P, ntok], F32, tag="pg")
                pu = psum.tile([P, ntok], F32, tag="pu")
                for k in range(KO_H):
                    nc.tensor.matmul(
                        pg[:], wg_m[:, k], xT_sb[:, k, off : off + ntok],
                        start=(k == 0), stop=(k == KO_H - 1),
                    )
                for k in range(KO_H):
                    nc.tensor.matmul(
                        pu[:], wu_m[:, k], xT_sb[:, k, off : off + ntok],
                        start=(k == 0), stop=(k == KO_H - 1),
                    )
                sg = tmp.tile([P, ntok], F32, tag="sg")
                nc.scalar.activation(
                    sg[:], pg[:], mybir.ActivationFunctionType.Silu
                )
                # ACT copy of up-psum so the DVE mul has a single-engine wait
                su = tmp.tile([P, ntok], F32, tag="su")
                nc.scalar.copy(su[:], pu[:])
                nc.vector.tensor_mul(
                    hT_sb[:, m, off : off + ntok], sg[:], su[:]
                )

            # Phase 2: out tiles [128 tok, 512 hid], contract over inter
            wd_sb = wdpool.tile([P, KO_I, HID], BF16, tag="wd")
            for k in range(KO_I):
                nc.scalar.dma_start(wd_sb[:, k], wds[s][:, k])
            for t in range(ntok // P):
                tg = off // P + t
                for n in range(n_nf):
                    po = psum2.tile([P, NF], F32, tag="po")
                    for k in range(KO_I):
                        nc.tensor.matmul(
                            po[:], hT_sb[:, k, off + t * P : off + (t + 1) * P],
                            wd_sb[:, k, bass.ts(n, NF)],
                            start=(k == 0), stop=(k == KO_I - 1),
                        )
                    ot = opool.tile([P, NF], BF16, tag="ot")
                    nc.vector.tensor_scalar_mul(
                        ot[:], po[:], probs_sb[:, tg : tg + 1]
                    )
                    nc.sync.dma_start(
                        out[bass.ts(tg, P), bass.ts(n, NF)], ot[:]
                    )
            off += ntok
    nc.finalize()
    return nc


# ---------------------------------------------------------------------------
# Host-side packing


def _pack_segments(counts):
    """Pack ragged per-expert token counts into a uniform per-core slot
    structure. Returns (scheme, assignment) where scheme is the per-core
    tuple of slot sizes and assignment[core] is a list of
    (expert, start_within_expert, n_real) per slot — or None if no listed
    scheme fits."""
    counts = [int(c) for c in counts]
    n_e = len(counts)

    schemes = [
        (384, 384, 256),
        (512, 384, 128),
        (512, 512, 128),
        (512, 384, 256),
        (384, 384, 384),
        (512, 512, 256),
        (512, 512, 384),
        (512, 512, 512),
        (512, 512, 512, 128),
        (512, 512, 512, 256),
        (512, 512, 512, 384),
        (512, 512, 512, 512),
    ]
    for scheme in schemes:
        sizes = sorted(set(scheme), reverse=True)
        cap = {sz: 8 * scheme.count(sz) for sz in sizes}

        # DFS over experts: choose per-expert usage of each slot class.
        def options(c):
            """All (usage-vector, waste) covering c, waste < min slot size."""
            opts = []
            maxn = [min(cap[sz], (c + sz - 1) // sz) for sz in sizes]

            def rec(i, rem, used):
                if rem <= 0:
                    opts.append(tuple(used) + (0,) * (len(sizes) - len(used)))
                    return
                if i == len(sizes):
                    return
                for n in range(maxn[i] + 1):
                    # a part must be "useful": last class can overshoot by
                    # less than one slot
                    used.append(n)
                    rec(i + 1, rem - n * sizes[i], used)
                    used.pop()

            rec(0, c, [])
            # keep non-dominated, low-waste options
            opts = sorted(
                set(opts),
                key=lambda u: (sum(x * sz for x, sz in zip(u, sizes)), sum(u)),
            )
            return opts[:24]

        expert_opts = [options(c) for c in counts]
        if any(not o for o in expert_opts):
            continue

        sol = [None] * n_e

        def dfs(e, caps):
            if e == n_e:
                return True
            for u in expert_opts[e]:
                if all(u[i] <= caps[i] for i in range(len(sizes))):
                    sol[e] = u
                    if dfs(e + 1, [caps[i] - u[i] for i in range(len(sizes))]):
                        return True
            sol[e] = None
            return False

        if not dfs(0, [cap[sz] for sz in sizes]):
            continue

        # Materialize parts per class, splitting each expert contiguously.
        parts = {sz: [] for sz in sizes}
        for e in range(n_e):
            pos = 0
            rem = counts[e]
            for i, sz in enumerate(sizes):
                for _ in range(sol[e][i]):
                    n_real = min(rem, sz)
                    if n_real > 0:
                        parts[sz].append((e, pos, n_real))
                        pos += n_real
                        rem -= n_real
                    else:
                        parts[sz].append((e, pos, 0))
        # Assign: core c takes the c-th part of each slot in scheme order.
        assignment = []
        idx = {sz: 0 for sz in sizes}
        ok = True
        for c in range(8):
            slots = []
            for sz in scheme:
                lst = parts[sz]
                i = idx[sz]
                if i < len(lst):
                    slots.append(lst[i])
                else:
                    slots.append((0, 0, 0))  # fully padded slot
                idx[sz] += 1
            assignment.append(slots)
        for sz in sizes:
            if idx[sz] < len(parts[sz]):
                ok = False  # parts left over: scheme infeasible
        if ok:
            return scheme, assignment
    return None, None


def _pack_x(x_pad, T):
    # xT[p, k, t] = x_pad[t, k*128+p]
    return np.ascontiguousarray(
        x_pad.T.reshape(KO_H, P, T).transpose(1, 0, 2)
    ).astype(NP_BF16)


_wcache: dict = {}


def _pack_weights(wg_e, wu_e, wd_e, key):
    if key in _wcache:
        return _wcache[key]
    # wg[p, m, k, i] = w_gate[k*128+p, m*128+i]
    wgt = np.ascontiguousarray(
        wg_e.reshape(KO_H, P, KO_I, P).transpose(1, 2, 0, 3)
    ).astype(NP_BF16)
    wut = np.ascontiguousarray(
        wu_e.reshape(KO_H, P, KO_I, P).transpose(1, 2, 0, 3)
    ).astype(NP_BF16)
    # wd[p, k, h] = w_down[k*128+p, h]
    wdt = np.ascontiguousarray(
        wd_e.reshape(KO_I, P, HID).transpose(1, 0, 2)
    ).astype(NP_BF16)
    _wcache[key] = (wgt, wut, wdt)
    return _wcache[key]


def _run(inputs, trace=False):
    x = np.asarray(inputs["permuted_x"], np.float32)
    probs = np.asarray(inputs["permuted_probs"], np.float32)
    wg = np.asarray(inputs["w_gate"], np.float32)
    wu = np.asarray(inputs["w_up"], np.float32)
    wd = np.asarray(inputs["w_down"], np.float32)
    counts = np.asarray(inputs["tokens_per_expert"]).astype(np.int64)
    offs = np.concatenate([[0], np.cumsum(counts)])
    assert offs[-1] == x.shape[0]

    _wcache.clear()

    scheme, assignment = _pack_segments(counts)
    if scheme is None:
        # Fallback: pad every core to the largest expert (always feasible).
        T = int(max(1, counts.max()))
        T = ((T + 511) // 512) * 512
        scheme = (T,)
        assignment = [[(e, 0, int(counts[e]))] for e in range(E)]

    key = tuple(scheme)
    if key not in _nc_cache:
        _nc_cache[key] = _build(key)
    nc = _nc_cache[key]

    T = sum(scheme)
    in_maps = []
    for c in range(E):
        x_pad = np.zeros((T, HID), np.float32)
        p_pad = np.zeros((T,), np.float32)
        im = {}
        off = 0
        for si, (sz, (e, pos, n)) in enumerate(zip(scheme, assignment[c])):
            g0 = int(offs[e]) + pos
            if n > 0:
                x_pad[off : off + n] = x[g0 : g0 + n]
                p_pad[off : off + n] = probs[g0 : g0 + n]
            wgt, wut, wdt = _pack_weights(wg[e], wu[e], wd[e], e)
            im[f"wg{si}"] = wgt
            im[f"wu{si}"] = wut
            im[f"wd{si}"] = wdt
            off += sz
        im["xT"] = _pack_x(x_pad, T)
        im["probs"] = np.ascontiguousarray(
            p_pad.reshape(T // P, P).T
        ).astype(np.float32)
        in_maps.append(im)

    res = run_bass_kernel_spmd(nc, in_maps, core_ids=list(range(E)), trace=trace)

    y = np.empty((x.shape[0], HID), np.float32)
    for c in range(E):
        out_c = np.asarray(res.results[c]["out"], dtype=np.float32)
        off = 0
        for sz, (e, pos, n) in zip(scheme, assignment[c]):
            if n > 0:
                g0 = int(offs[e]) + pos
                y[g0 : g0 + n] = out_c[off : off + n]
            off += sz
    return y, res


def kernel(**inputs) -> np.ndarray:
    y, _ = _run(inputs, trace=False)
    return y
